# revision 52
# baseline (speedup 1.0000x reference)
"""BotRGCN Trainium2 kernel (8 NeuronCores, SPMD) — v2.

Design: nodes row-wise across 8 cores (12800 padded rows/core). The RGCN
aggregation avoids dma_scatter_add entirely: edges are dst-sorted into a
static slot grid (quotas shared across cores), dma_gather pulls bf16
node rows (256-col padded) from an all-gathered table, and segment sums
run as one-hot matmuls on the tensor engine accumulating in PSUM per
(dst-block, relation). x stays feature-major in SBUF between layers.
Host preprocessing is structural only (sharding, slotting, counts).

Self-contained: hardcodes N=100000, E=1600000, EMB=192, 2 relations.
"""

import os
import sys
from contextlib import ExitStack

import numpy as np

for _p in ("/opt/trn_rl_repo",):
    if os.path.isdir(_p) and _p not in sys.path:
        sys.path.insert(0, _p)

import concourse.bass as bass
import concourse.mybir as mybir
from concourse import bacc, library_config, tile
from concourse.bass_utils import run_bass_kernel_spmd

F32 = mybir.dt.float32
B16 = mybir.dt.bfloat16
F16 = mybir.dt.float16
I16 = mybir.dt.int16
AX = mybir.AluOpType
ACTF = mybir.ActivationFunctionType

LEAKY = 0.01
P = 128
EMB = 192
TH = 64
D_DES = 768
KD = D_DES // P            # 6
NCORES = 8
N_PER = 12500              # real nodes per core
NSH = 12800                # padded nodes per core
N_TOT = NCORES * NSH       # 102400 = 4 * 25600
CHUNK = 25600              # gather window rows (int16 reach)
NCHUNK = 4
NBLK = NSH // P            # 100 dst blocks per core
NSB = NBLK // 4            # 25 superblocks (512 nodes = dense chunk)
PADF = 256                 # padded feature columns in gather table
PAD_ID = 600.0             # sel sentinel (outside [0,512), bf16-exact)


# ----------------------------------------------------------------------------
# Host-side structural preprocessing
# ----------------------------------------------------------------------------

def _wrap16(a):
    w = a.reshape(-1, 16).T.astype(np.int16)  # [16, L/16]
    return np.ascontiguousarray(np.tile(w, (8, 1)))


def build_schedule(edge_index, edge_type):
    """Static slot grid + per-core slot arrays.

    Returns (sched, per_core) where sched is structural metadata shared by
    the single SPMD program and per_core has gidx/id2/inv2 arrays.
    """
    ei = np.asarray(edge_index).astype(np.int64)
    et = np.asarray(edge_type).astype(np.int64)
    src, dst = ei[0], ei[1]
    # table layout: [quarter q][core i][row n%3200] so chunk c == the output
    # of quarter-collective c (lets chunk-c gathers start after 1/4 of the
    # allgather instead of all of it)
    QR = NSH // NCHUNK                      # 3200 rows per core per quarter
    s_core = src // N_PER
    s_loc = src % N_PER
    c_all = (s_loc // QR).astype(np.int32)
    lidx_all = (s_core * QR + (s_loc % QR)).astype(np.int32)
    dst_core = (dst // N_PER).astype(np.int32)
    dst_local = (dst % N_PER).astype(np.int32)
    b_all = (dst_local // P).astype(np.int32)

    # counts n[core, b, c] (and per-rel) for quotas and dead-matmul pruning
    key = ((dst_core.astype(np.int64) * NBLK + b_all) * NCHUNK + c_all)
    cnt = np.bincount(key, minlength=NCORES * NBLK * NCHUNK).reshape(
        NCORES, NBLK, NCHUNK)
    cnt_r = np.bincount(key * 2 + et, minlength=NCORES * NBLK * NCHUNK * 2
                        ).reshape(NCORES, NBLK, NCHUNK, 2)
    Q = cnt.max(axis=0)
    Q = ((Q + 31) // 32) * 32          # [NBLK, NCHUNK]

    # batches in (SB, c) order. PSUM accumulation groups must not interleave
    # within a bank, so each batch closes its groups (per-batch start/stop)
    # and the cross-batch accumulation happens in an SBUF fp32 tile.
    batches = []   # per batch: dict(n, units, mms, accs)
    off16 = 0
    off128 = 0
    seen_tiles = set()   # (SB, bi, r) with at least one matmul so far
    for SB in range(NSB):
        for c in range(NCHUNK):
            units = []
            o = 0
            for bi in range(4):
                b = SB * 4 + bi
                q = int(Q[b, c])
                if q == 0:
                    continue
                units.append((b, o, q))
                o += q
            n = ((o + 127) // 128) * 128
            mms = []
            first_in_batch = {}
            last_in_batch = {}
            for g in range(n // 128):
                lo, hi = g * P, g * P + P
                for (b, uo, q) in units:
                    if uo < hi and uo + q > lo:
                        bi = b - SB * 4
                        par = b % 2
                        # prune matmuls whose rel-span (any core) misses g
                        r0max = int(cnt_r[:, b, c, 0].max())
                        for r in range(2):
                            if r == 0:
                                rl, rh = uo, uo + r0max
                            else:
                                rmin = int(cnt_r[:, b, c, 0].min())
                                rl = uo + rmin
                                rh = uo + int(
                                    (cnt_r[:, b, c, 0]
                                     + cnt_r[:, b, c, 1]).max())
                            if rl >= hi or rh <= lo:
                                continue
                            k = (bi, r)
                            idx = len(mms)
                            mms.append([g, bi, r, par, False, False])
                            if k not in first_in_batch:
                                first_in_batch[k] = idx
                            last_in_batch[k] = idx
            for k, idx in first_in_batch.items():
                mms[idx][4] = True
            for k, idx in last_in_batch.items():
                mms[idx][5] = True
            # accumulate-into-SBUF ops after this batch
            accs = []
            for (bi, r) in sorted(first_in_batch.keys()):
                key = (SB, bi, r)
                accs.append((bi, r, key not in seen_tiles))
                seen_tiles.add(key)
            batches.append(dict(SB=SB, c=c, n=n, units=units, mms=mms,
                                accs=accs, off16=off16, off128=off128))
            off16 += n // 16
            off128 += n // 128
    tot = off16 * 16
    sched = dict(batches=batches, tot=tot, touched=seen_tiles)

    # per-core slot arrays (vectorized fill)
    unit_base = np.full((NBLK, NCHUNK), -1, np.int64)
    for bt in batches:
        for (b, uo, q) in bt["units"]:
            unit_base[b, bt["c"]] = bt["off16"] * 16 + uo
    per_core = []
    for i in range(NCORES):
        m_core = dst_core == i
        e_b = b_all[m_core]
        e_c = c_all[m_core]
        e_r = et[m_core]
        e_l = lidx_all[m_core]
        e_d = dst_local[m_core]
        order = np.lexsort((e_r, e_c, e_b))
        gk = (e_b.astype(np.int64) * NCHUNK + e_c)[order]
        ne = len(gk)
        starts = np.flatnonzero(np.r_[True, gk[1:] != gk[:-1]])
        counts = np.diff(np.r_[starts, ne])
        pos = np.arange(ne) - np.repeat(starts, counts)
        slot = unit_base[e_b[order], e_c[order]] + pos
        gvals = np.zeros(tot, np.int32)
        ivals = np.full(tot, PAD_ID, np.float32)
        gvals[slot] = e_l[order]
        ivals[slot] = (e_d[order] % P) + P * (
            2 * (e_b[order] % 2) + e_r[order])
        gidx = _wrap16(gvals)
        id2 = np.ascontiguousarray(
            ivals.reshape(-1, P).T.astype(np.float16))     # [128, tot/128]

        cnt2 = np.bincount(2 * dst_local[m_core] + et[m_core],
                           minlength=2 * N_PER)
        inv_full = np.ones(2 * NSH, np.float32)
        inv_full[:2 * N_PER] = 1.0 / np.maximum(cnt2, 1)
        # layout [128, NSB, 4, 2]: p, SB, bi, r  (dst = (SB*4+bi)*128 + p)
        inv = inv_full.reshape(NSH, 2)[
            np.arange(NSH).reshape(NBLK, P), :]            # [NBLK, P, 2]
        inv = inv.reshape(NSB, 4, P, 2).transpose(2, 0, 1, 3)
        per_core.append(dict(gidx=gidx, id2=id2,
                             inv2=np.ascontiguousarray(inv.reshape(P, -1))))
    return sched, per_core


def prep_inputs(inputs):
    des = np.asarray(inputs["des"], np.float32)
    num_prop = np.asarray(inputs["num_prop"], np.float32)
    cat_prop = np.asarray(inputs["cat_prop"], np.float32)

    w_rel = np.asarray(inputs["W_rel"], np.float32)
    shared = {
        "w_des": np.asarray(inputs["W_des"], np.float32),
        "w_num": np.asarray(inputs["W_num"], np.float32),
        "w_cat": np.asarray(inputs["W_cat"], np.float32),
        "b0": np.concatenate([np.asarray(inputs["b_des"], np.float32),
                              np.asarray(inputs["b_num"], np.float32),
                              np.asarray(inputs["b_cat"], np.float32)]),
        "w_in": np.asarray(inputs["W_in"], np.float32),
        "b_in": np.asarray(inputs["b_in"], np.float32),
        "w_root": np.asarray(inputs["W_root"], np.float32),
        "w_rel0": np.ascontiguousarray(w_rel[0]),
        "w_rel1": np.ascontiguousarray(w_rel[1]),
        "b_rgcn": np.asarray(inputs["b_rgcn"], np.float32),
        "w_o1": np.asarray(inputs["W_o1"], np.float32),
        "b_o1": np.asarray(inputs["b_o1"], np.float32),
        "w_o2": np.asarray(inputs["W_o2"], np.float32),
        "b_o2r": np.tile(np.asarray(inputs["b_o2"], np.float32)[None, :],
                         (P, 1)),
    }

    sched, per_core = build_schedule(inputs["edge_index"],
                                     inputs["edge_type"])
    in_maps = []
    for i in range(NCORES):
        rs, re = i * N_PER, (i + 1) * N_PER
        desT = np.zeros((D_DES, NSH), np.float32)
        desT[:, :N_PER] = des[rs:re].T
        numT = np.zeros((4, NSH), np.float32)
        numT[:, :N_PER] = num_prop[rs:re].T
        catT = np.zeros((3, NSH), np.float32)
        catT[:, :N_PER] = cat_prop[rs:re].T
        in_maps.append(dict(shared, desT=desT, numT=numT, catT=catT,
                            **per_core[i]))
    return sched, in_maps


# ----------------------------------------------------------------------------
# Device program
# ----------------------------------------------------------------------------

def build_program(sched):
    nc = bacc.Bacc(None, num_devices=NCORES)
    TOT = sched["tot"]

    def param(name, shape, dtype=F32, out=False):
        return nc.declare_dram_parameter(name, list(shape), dtype, isOutput=out)

    desT = param("desT", (D_DES, NSH))
    numT = param("numT", (4, NSH))
    catT = param("catT", (3, NSH))
    gidx = param("gidx", (P, TOT // 16), I16)
    id2d = param("id2", (P, TOT // P), F16)
    inv2 = param("inv2", (P, NSB * 4 * 2))
    w_des = param("w_des", (D_DES, TH))
    w_num = param("w_num", (4, TH))
    w_cat = param("w_cat", (3, TH))
    b0 = param("b0", (EMB,))
    w_in = param("w_in", (EMB, EMB))
    b_in = param("b_in", (EMB,))
    w_root = param("w_root", (EMB, EMB))
    w_rel0 = param("w_rel0", (EMB, EMB))
    w_rel1 = param("w_rel1", (EMB, EMB))
    b_rgcn = param("b_rgcn", (EMB,))
    w_o1 = param("w_o1", (EMB, EMB))
    b_o1 = param("b_o1", (EMB,))
    w_o2 = param("w_o2", (EMB, 2))
    b_o2r = param("b_o2r", (P, 2))
    out_p = param("out", (NSH, 2), out=True)
    dbg_x1 = param("dbg_x1", (P, PADF), out=True)
    dbg_agg = param("dbg_agg", (P, EMB), out=True)
    dbg_agt = param("dbg_agt", (P, P), out=True)
    dbg_x2 = param("dbg_x2", (P, P), out=True)
    dbg_xg2 = param("dbg_xg2", (P, PADF), out=True)
    dbg_agg2 = param("dbg_agg2", (P, EMB), out=True)

    QR = NSH // NCHUNK
    agin1 = [nc.dram_tensor(f"agin1q{q}", [QR, PADF], B16)
             for q in range(NCHUNK)]
    xg1 = nc.dram_tensor("xg1", [N_TOT, PADF], B16, addr_space="Shared")
    agin2 = [nc.dram_tensor(f"agin2q{q}", [QR, PADF], B16)
             for q in range(NCHUNK)]
    xg2 = nc.dram_tensor("xg2", [N_TOT, PADF], B16, addr_space="Shared")

    replica = [list(range(NCORES))]

    with tile.TileContext(nc) as tc, ExitStack() as top:
        nc.gpsimd.load_library(library_config.mlp)
        const = top.enter_context(tc.tile_pool(name="const", bufs=1))

        def cload(src_ap, shape, name, dtype=F32):
            t = const.tile(list(shape), dtype, tag=name)
            nc.sync.dma_start(out=t[:], in_=src_ap)
            return t

        ident_dram = nc.inline_tensor(np.eye(P, dtype=np.float32),
                                      name="identity128")
        ident = cload(ident_dram[:, :], (P, P), "ident")
        iota_dram = nc.inline_tensor(
            np.tile(np.arange(512, dtype=np.float16), (P, 1)), name="iota512")
        iota = cload(iota_dram[:, :], (P, 512), "iota", dtype=F16)

        def bcast(w, name):
            # fp32 load + bf16 cast; rows split 0:128 / 128:end
            rows = int(w.shape[0])
            cols = int(w.shape[1])
            a32 = cload(w[0:P, :], (P, cols), name + "A32")
            a = const.tile([P, cols], B16, tag=name + "A")
            nc.vector.tensor_copy(a[:], a32[:])
            if rows > P:
                b32 = cload(w[P:rows, :], (rows - P, cols), name + "B32")
                b = const.tile([rows - P, cols], B16, tag=name + "B")
                nc.vector.tensor_copy(b[:], b32[:])
                return a, b
            return a, None

        wdes = cload(w_des[:, :].rearrange("(k p) m -> p k m", p=P),
                     (P, KD, TH), "wdes")
        wnum = cload(w_num[:, :], (4, TH), "wnum")
        wcat = cload(w_cat[:, :], (3, TH), "wcat")
        winA, winB = bcast(w_in, "win")
        wrootA, wrootB = bcast(w_root, "wroot")
        wrel0A, wrel0B = bcast(w_rel0, "wrel0")
        wrel1A, wrel1B = bcast(w_rel1, "wrel1")
        wrelA = [wrel0A, wrel1A]
        wrelB = [wrel0B, wrel1B]
        wo1A, wo1B = bcast(w_o1, "wo1")
        wo2A32 = cload(w_o2[0:P, :], (P, 2), "wo2A32")
        wo2B32 = cload(w_o2[P:EMB, :], (EMB - P, 2), "wo2B32")
        wo2A = const.tile([P, 2], B16, tag="wo2A")
        wo2B = const.tile([EMB - P, 2], B16, tag="wo2B")
        nc.vector.tensor_copy(wo2A[:], wo2A32[:])
        nc.vector.tensor_copy(wo2B[:], wo2B32[:])

        def load_colvec(v, name):
            a = const.tile([P, 1], F32, tag=name + "A")
            b = const.tile([EMB - P, 1], F32, tag=name + "B")
            nc.sync.dma_start(out=a[:], in_=v[0:P].unsqueeze(1))
            nc.sync.dma_start(out=b[:], in_=v[P:EMB].unsqueeze(1))
            return a, b

        b0A, b0B = load_colvec(b0, "b0")
        binA, binB = load_colvec(b_in, "bin")
        brgA, brgB = load_colvec(b_rgcn, "brg")
        bo1A, bo1B = load_colvec(b_o1, "bo1")
        bo2 = cload(b_o2r[:, :], (P, 2), "bo2")

        inv_sb = const.tile([P, NSB, 4, 2], F32, tag="inv")
        nc.sync.dma_start(
            out=inv_sb[:],
            in_=inv2[:, :].rearrange("p (s b r) -> p s b r", s=NSB, b=4, r=2))

        outsb = const.tile([P, NSH // P, 2], F32, tag="outsb")
        # persistent feature-major x (row 0: feats 0:128, row 1: feats 128:192)
        xt_all = const.tile([P, 2, NSH], B16, tag="xt_all")

        def leaky_(ap):
            nc.vector.scalar_tensor_tensor(ap, ap, LEAKY, ap,
                                           op0=AX.mult, op1=AX.max)

        def act_bias(out_ap, in_ap, bias_ap):
            nc.scalar.activation(out_ap, in_ap, ACTF.Identity,
                                 bias=bias_ap, scale=1.0)

        def mm(out_ap, l_ap, r_ap, first=False, last=False):
            nc.tensor.matmul(out_ap, l_ap, r_ap, start=first, stop=last)

        def emit_node_major(pool, psum, xa_f32, xb_f32, dst_quads, sbi):
            # x chunk ([128,512]+[64,512] fp32) -> node-major bf16 [512,256]
            # into per-quarter tensors (splits at t boundaries: 3200%128==0)
            agsb = pool.tile([P, 4, PADF], B16, tag="agsb")
            nc.vector.memset(agsb[:, :, EMB:PADF], 0.0)
            for t in range(4):
                pt = psum.tile([P, EMB], F32, tag="pt")
                nc.tensor.matmul(pt[:, 0:P],
                                 xa_f32[:, bass.ts(t, P)], ident[:],
                                 is_transpose=True)
                nc.tensor.matmul(pt[:, P:EMB],
                                 xb_f32[:, bass.ts(t, P)], ident[0:TH, 0:TH],
                                 is_transpose=True)
                nc.vector.tensor_copy(agsb[:, t, 0:EMB], pt[:])
            t0 = 0
            while t0 < 4:
                row0 = sbi * 512 + t0 * P
                q = row0 // QR
                t1 = min(4, ((q + 1) * QR - sbi * 512) // P)
                nc.sync.dma_start(
                    out=dst_quads[q][row0 - q * QR:
                                     row0 - q * QR + (t1 - t0) * P,
                                     :].rearrange("(t p) f -> p t f", p=P),
                    in_=agsb[:, t0:t1, :])
                t0 = t1

        # ------------------------------------------------------------
        # Phase A: x1 = leaky(leaky(feats @ W_*) @ W_in + b_in)
        # ------------------------------------------------------------
        with tc.tile_pool(name="pa", bufs=2) as pa, \
             tc.tile_pool(name="paps", bufs=1, space="PSUM") as paps, \
             tc.tile_pool(name="panp", bufs=2) as panp:
            desT_r = desT[:, :].rearrange("(k p) n -> p k n", p=P)

            for sbi in range(NSB):
                sl = bass.ts(sbi, 512)
                np_sb = panp.tile([4, 512], F32, tag="np")
                cp_sb = panp.tile([3, 512], F32, tag="cp")
                nc.sync.dma_start(out=np_sb[:], in_=numT[:, sl])
                nc.sync.dma_start(out=cp_sb[:], in_=catT[:, sl])
                des_c = pa.tile([P, KD, 512], F32, tag="des")
                nc.sync.dma_start(out=des_c[:], in_=desT_r[:, :, sl])
                psA = paps.tile([P, 512], F32, tag="psA")
                psB = paps.tile([TH, 512], F32, tag="psB")
                for k in range(KD):
                    nc.tensor.matmul(psA[0:TH, :], wdes[:, k, :],
                                     des_c[:, k, :],
                                     start=(k == 0), stop=(k == KD - 1))
                nc.tensor.matmul(psA[TH:P, :], wnum[:], np_sb[:],
                                 start=True, stop=True)
                nc.tensor.matmul(psB[:], wcat[:], cp_sb[:],
                                 start=True, stop=True)
                x0A = pa.tile([P, 512], F32, tag="x0A")
                x0B = pa.tile([TH, 512], F32, tag="x0B")
                act_bias(x0A[:], psA[:], b0A[:])
                act_bias(x0B[:], psB[:], b0B[:])
                leaky_(x0A[:])
                leaky_(x0B[:])
                x0Ab = pa.tile([P, 512], B16, tag="x0Ab")
                x0Bb = pa.tile([TH, 512], B16, tag="x0Bb")
                nc.vector.tensor_copy(x0Ab[:], x0A[:])
                nc.vector.tensor_copy(x0Bb[:], x0B[:])
                ps1A = paps.tile([P, 512], F32, tag="ps1A")
                ps1B = paps.tile([TH, 512], F32, tag="ps1B")
                mm(ps1A[:], winA[:, 0:P], x0Ab[:], first=True)
                mm(ps1A[:], winB[:, 0:P], x0Bb[:], last=True)
                mm(ps1B[:], winA[:, P:EMB], x0Ab[:], first=True)
                mm(ps1B[:], winB[:, P:EMB], x0Bb[:], last=True)
                x1A = pa.tile([P, 512], F32, tag="x1A")
                x1B = pa.tile([TH, 512], F32, tag="x1B")
                act_bias(x1A[:], ps1A[:], binA[:])
                act_bias(x1B[:], ps1B[:], binB[:])
                leaky_(x1A[:])
                leaky_(x1B[:])
                nc.vector.tensor_copy(xt_all[:, 0, sl], x1A[:])
                nc.vector.tensor_copy(xt_all[0:TH, 1, sl], x1B[:])
                emit_node_major(pa, paps, x1A[:], x1B[:], agin1, sbi)

        # ------------------------------------------------------------
        # RGCN layers
        # ------------------------------------------------------------
        batches = sched["batches"]
        touched = sched["touched"]

        def layer(lid, xg, consume):
            with tc.tile_pool(name=f"gb{lid}", bufs=3) as gb, \
                 tc.tile_pool(name=f"gi{lid}", bufs=4) as gip, \
                 tc.tile_pool(name=f"sp{lid}", bufs=2) as sp, \
                 tc.tile_pool(name=f"agps{lid}", bufs=1, space="PSUM") as agps, \
                 tc.tile_pool(name=f"tp{lid}", bufs=1, space="PSUM") as tp, \
                 tc.tile_pool(name=f"ag{lid}", bufs=2) as ag, \
                 tc.tile_pool(name=f"mmps{lid}", bufs=1, space="PSUM") as mmps, \
                 tc.tile_pool(name=f"cons{lid}", bufs=2) as cpool:
                bidx = 0
                for SB in range(NSB):
                    # per-batch PSUM groups (r-major: banks never interleave
                    # two open groups), fp32 SBUF accumulator across chunks
                    aggp = agps.tile([P, 8, PADF], F32, tag="agg")
                    acc = ag.tile([P, 8, EMB], F32, tag="acc")
                    nc.vector.memset(acc[:], 0.0)
                    for c in range(NCHUNK):
                        bt_meta = batches[bidx]
                        bidx += 1
                        n = bt_meta["n"]
                        if n == 0:
                            continue
                        ng = n // 128
                        o16 = bt_meta["off16"]
                        o128 = bt_meta["off128"]
                        gi = gip.tile([P, n // 16], I16, tag="gi")
                        nc.sync.dma_start(out=gi[:],
                                          in_=gidx[:, o16:o16 + n // 16])
                        idt = gip.tile([P, ng], F16, tag="idt")
                        nc.sync.dma_start(out=idt[:],
                                          in_=id2d[:, o128:o128 + ng])
                        bt = gb.tile([P, ng, PADF], B16, tag="bt")
                        nc.gpsimd.dma_gather(
                            bt[:], xg[c * CHUNK:(c + 1) * CHUNK, :], gi[:],
                            n, n, PADF, single_packet=False)
                        sel = sp.tile([P, ng, 512], B16, tag="sel")
                        nc.vector.tensor_tensor(
                            sel[:],
                            iota[:].unsqueeze(1).broadcast_to([P, ng, 512]),
                            idt[:].unsqueeze(-1).broadcast_to([P, ng, 512]),
                            op=AX.is_equal)
                        for (g, bi, r, par, first, last) in bt_meta["mms"]:
                            col = 256 * par + 128 * r
                            mm(aggp[:, r * 4 + bi, 0:EMB],
                               sel[:, g, col:col + 128],
                               bt[:, g, 0:EMB], first=first, last=last)
                        if len(bt_meta["accs"]) == 8:
                            nc.vector.tensor_tensor(
                                acc[:], acc[:], aggp[:, :, 0:EMB], op=AX.add)
                        else:
                            for (bi, r, _f) in bt_meta["accs"]:
                                nc.vector.tensor_tensor(
                                    acc[:, r * 4 + bi, :],
                                    acc[:, r * 4 + bi, :],
                                    aggp[:, r * 4 + bi, 0:EMB], op=AX.add)
                    # mean + transpose to feature-major aggT
                    aggT_a = [ag.tile([P, 512], B16, tag=f"aTa{r}",
                                      name=f"aTa{r}") for r in range(2)]
                    aggT_b = [ag.tile([TH, 512], B16, tag=f"aTb{r}",
                                      name=f"aTb{r}") for r in range(2)]
                    for r in range(2):
                        for bi in range(4):
                            key = (SB, bi, r)
                            anm = ag.tile([P, EMB], F32, tag="anm")
                            if key in touched:
                                nc.vector.tensor_tensor(
                                    anm[:], acc[:, r * 4 + bi, :],
                                    inv_sb[:, SB, bi, r].unsqueeze(-1)
                                    .broadcast_to([P, EMB]), op=AX.mult)
                            else:
                                nc.vector.memset(anm[:], 0.0)
                            pst = tp.tile([P, 256], F32, tag="pst")
                            nc.tensor.matmul(pst[:, 0:P], anm[:, 0:P],
                                             ident[:], is_transpose=True)
                            nc.tensor.matmul(pst[0:TH, P:256], anm[:, P:EMB],
                                             ident[:], is_transpose=True)
                            nc.vector.tensor_copy(
                                aggT_a[r][:, bass.ts(bi, P)], pst[:, 0:P])
                            nc.vector.tensor_copy(
                                aggT_b[r][:, bass.ts(bi, P)],
                                pst[0:TH, P:256])
                            if SB == 0 and bi == 0 and r == 0:
                                dsb = ag.tile([P, EMB], F32, tag="dbga")
                                nc.vector.tensor_copy(dsb[:], anm[:])
                                nc.sync.dma_start(
                                    out=(dbg_agg if lid == 1
                                         else dbg_agg2)[:, :],
                                    in_=dsb[:])
                    if lid == 1 and SB == 0:
                        dst2 = ag.tile([P, P], F32, tag="dbgt")
                        nc.vector.tensor_copy(dst2[:], aggT_a[0][:, 0:P])
                        nc.sync.dma_start(out=dbg_agt[:, :], in_=dst2[:])
                    # dense: root + relations
                    sl = bass.ts(SB, 512)
                    oA = mmps.tile([P, 512], F32, tag="oA")
                    oB = mmps.tile([TH, 512], F32, tag="oB")
                    mm(oA[:], wrootA[:, 0:P], xt_all[:, 0, sl], first=True)
                    mm(oA[:], wrootB[:, 0:P], xt_all[0:TH, 1, sl])
                    mm(oB[:], wrootA[:, P:EMB], xt_all[:, 0, sl], first=True)
                    mm(oB[:], wrootB[:, P:EMB], xt_all[0:TH, 1, sl])
                    for r in range(2):
                        last = (r == 1)
                        mm(oA[:], wrelA[r][:, 0:P], aggT_a[r][:])
                        mm(oA[:], wrelB[r][:, 0:P], aggT_b[r][:], last=last)
                        mm(oB[:], wrelA[r][:, P:EMB], aggT_a[r][:])
                        mm(oB[:], wrelB[r][:, P:EMB], aggT_b[r][:], last=last)
                    consume(SB, oA, oB, cpool, mmps)

        # ---- layer 1 ----
        for q in range(NCHUNK):
            nc.gpsimd.collective_compute(
                "AllGather", AX.bypass, replica_groups=replica,
                ins=[agin1[q][:, :].opt()],
                outs=[xg1[q * CHUNK:(q + 1) * CHUNK, :].opt()])
        with tc.tile_pool(name="dbg", bufs=1) as dbgp:
            dt1 = dbgp.tile([P, PADF], B16, tag="d1")
            nc.sync.dma_start(out=dt1[:], in_=xg1[0:P, :])
            dt1f = dbgp.tile([P, PADF], F32, tag="d1f")
            nc.vector.tensor_copy(dt1f[:], dt1[:])
            nc.sync.dma_start(out=dbg_x1[:, :], in_=dt1f[:])

        def consume1(SB, oA, oB, cpool, cpps):
            sl = bass.ts(SB, 512)
            x2a = cpool.tile([P, 512], F32, tag="x2a")
            x2b = cpool.tile([TH, 512], F32, tag="x2b")
            act_bias(x2a[:], oA[:], brgA[:])
            act_bias(x2b[:], oB[:], brgB[:])
            if SB == 0:
                dx2 = cpool.tile([P, P], F32, tag="dx2")
                nc.vector.tensor_copy(dx2[:], x2a[:, 0:P])
                nc.sync.dma_start(out=dbg_x2[:, :], in_=dx2[:])
            nc.scalar.activation(xt_all[:, 0, sl], x2a[:], ACTF.Identity)
            nc.scalar.activation(xt_all[0:TH, 1, sl], x2b[:], ACTF.Identity)
            emit_node_major(cpool, cpps, x2a[:], x2b[:], agin2, SB)

        layer(1, xg1, consume1)

        # ---- layer 2 (+ fused output head) ----
        for q in range(NCHUNK):
            nc.gpsimd.collective_compute(
                "AllGather", AX.bypass, replica_groups=replica,
                ins=[agin2[q][:, :].opt()],
                outs=[xg2[q * CHUNK:(q + 1) * CHUNK, :].opt()])
        with tc.tile_pool(name="dbg2", bufs=1) as dbgp2:
            dt2 = dbgp2.tile([P, PADF], B16, tag="d2")
            nc.sync.dma_start(out=dt2[:], in_=xg2[0:P, :])
            dt2f = dbgp2.tile([P, PADF], F32, tag="d2f")
            nc.vector.tensor_copy(dt2f[:], dt2[:])
            nc.sync.dma_start(out=dbg_xg2[:, :], in_=dt2f[:])

        def consume2(SB, oA, oB, cpool, cpps):
            x3A = cpool.tile([P, 512], B16, tag="x3A")
            x3B = cpool.tile([TH, 512], B16, tag="x3B")
            act_bias(x3A[:], oA[:], brgA[:])
            act_bias(x3B[:], oB[:], brgB[:])
            # reuse the oA/oB banks (bufs=1 pool; WAR deps order the reuse)
            p1A = cpps.tile([P, 512], F32, tag="oA")
            p1B = cpps.tile([TH, 512], F32, tag="oB")
            mm(p1A[:], wo1A[:, 0:P], x3A[:], first=True)
            mm(p1A[:], wo1B[:, 0:P], x3B[:], last=True)
            mm(p1B[:], wo1A[:, P:EMB], x3A[:], first=True)
            mm(p1B[:], wo1B[:, P:EMB], x3B[:], last=True)
            o1A = cpool.tile([P, 512], F32, tag="o1A")
            o1B = cpool.tile([TH, 512], F32, tag="o1B")
            act_bias(o1A[:], p1A[:], bo1A[:])
            act_bias(o1B[:], p1B[:], bo1B[:])
            leaky_(o1A[:])
            leaky_(o1B[:])
            o1Ab = cpool.tile([P, 512], B16, tag="o1Ab")
            o1Bb = cpool.tile([TH, 512], B16, tag="o1Bb")
            nc.scalar.activation(o1Ab[:], o1A[:], ACTF.Identity)
            nc.scalar.activation(o1Bb[:], o1B[:], ACTF.Identity)
            for t in range(4):
                pso = cpps.tile([P, 2], F32, tag="pt")
                mm(pso[:], o1Ab[:, bass.ts(t, P)], wo2A[:], first=True)
                mm(pso[:], o1Bb[:, bass.ts(t, P)], wo2B[:], last=True)
                nc.vector.tensor_tensor(outsb[:, SB * 4 + t, :], pso[:],
                                        bo2[:], op=AX.add)

        layer(2, xg2, consume2)

        nc.sync.dma_start(
            out=out_p[:, :].rearrange("(t p) c -> p t c", p=P),
            in_=outsb[:])

    return nc


def _install_ntff_shim():
    import types
    try:
        import antenv.axon_hooks  # noqa: F401
        return
    except ImportError:
        pass
    try:
        from trn_agent_boot.trn_boot import _ntff_profile_via_ctypes
        hook = _ntff_profile_via_ctypes("/opt/axon/libaxon_pjrt.so")
    except Exception:
        hook = None
    mod = types.ModuleType("antenv.axon_hooks")
    mod.get_axon_ntff_profile_hook = lambda: hook
    mod.set_axon_ntff_profile_hook = lambda h: None
    sys.modules["antenv.axon_hooks"] = mod


def run_on_hw(inputs, trace=False, trace_kwargs=None):
    if trace:
        _install_ntff_shim()
    sched, in_maps = prep_inputs(inputs)
    nc = build_program(sched)
    nc.finalize()
    res = run_bass_kernel_spmd(nc, in_maps, list(range(NCORES)),
                               trace=trace, **(trace_kwargs or {}))
    outs = [res.results[i]["out"][:N_PER] for i in range(NCORES)]
    full = np.concatenate(outs, axis=0)
    return full, res


def kernel(**inputs):
    full, _ = run_on_hw(inputs, trace=False)
    return full
